# revision 1
# baseline (speedup 1.0000x reference)
"""FCCaps (EfficientCapsNet fully-connected capsule routing) Trainium2 kernel.

Problem:  U_l (64, 512, 16) f32, W (512, 32, 16, 32) f32
    U_hat = einsum('bij,ikjl->bikl', U_l, W)          (B, n_l, n_h, d_h)
    A     = einsum('bikl,bhkl->bhik', U_hat, U_hat)
    C     = softmax(sum_i A / sqrt(d_h), axis=-1)      (B, n_l, n_h)
    U_h   = einsum('bikl,bik->bkl', U_hat, C)          (B, n_h, d_h)
    out   = squash(U_h)

Key algebraic identity used:
    A_sum[b,h,k] = sum_l U_hat[b,h,k,l] * S[b,k,l],  S = sum_i U_hat
so the (B, n_l, n_l, n_h) agreement tensor is never materialized.

Distribution: shard n_l (the i axis) 8 ways.  Each core computes U_hat for its
64 i's and all 64 batches, a partial S (AllReduce, 256KB), local softmax
(k is fully local), partial U_h (ReduceScatter over batch), then squash of its
own 8-batch output slice.  Host concatenates the 8 slices.

Host-side prep is layout-only (transpose/reshape/zero-fill of inputs).
"""

import sys

sys.path.insert(0, "/opt/trn_rl_repo")

import numpy as np

import concourse.bass as bass
import concourse.mybir as mybir
import concourse.tile as tile
from concourse import bacc
from concourse.bass_utils import run_bass_kernel_spmd

F32 = mybir.dt.float32
F32R = mybir.dt.float32r
AX = mybir.AxisListType
OP = mybir.AluOpType
AF = mybir.ActivationFunctionType

B, NL, NH, DL, DH = 64, 512, 32, 16, 32
NCORES = 8
ILOC = NL // NCORES          # 64 i's per core
NG = ILOC // 8               # 8 groups of 8 i_sub
NCB = B // 16                # 4 batch chunks of 16
KL = NH * DH                 # 1024
ATT = 5.656854249492381      # sqrt(d_h)
EPS = 1e-20

_CACHE = {}


def _bcast(ap, n):
    """Append a step-0 innermost dim (read-broadcast) to an AP."""
    return bass.AP(ap.tensor, ap.offset, list(ap.ap) + [[0, n]])


def _r(ap):
    """Reinterpret an fp32 AP as float32r for the PE: same bits, but the
    tensor engine streams it at 1 cycle/row instead of fp32's 4 (for moving
    free dim >= 256)."""
    return ap.bitcast(F32R)


def build_program():
    nc = bacc.Bacc(
        "TRN2",
        target_bir_lowering=False,
        debug=False,
        enable_asserts=False,
        num_devices=NCORES,
    )

    # WUL[g] = concat(Wr[g] (1024 cols), UlT[g] (64), Lb[g,0..3] (4x128)) so one
    # DMA covers all matmul operands of a group (keeps <=1 sem wait per matmul;
    # walrus rejects LDWEIGHTS with 2 waits).
    WUL = nc.dram_tensor("WUL", [NG, 128, 1600], F32, kind="ExternalInput")
    Ones = nc.dram_tensor("Ones", [NCB, 128, B], F32, kind="ExternalInput")
    # Bcast[cb][b, i_sub*16+br] = (b == 16*cb+br): PE-side partition broadcast
    # of S across the 8 i_sub replicas (a DMA to 16 partitions is ~8x slower)
    Bcast = nc.dram_tensor("Bcast", [NCB, B, 128], F32, kind="ExternalInput")
    out_sh = nc.dram_tensor("out_sh", [B // NCORES, KL], F32, kind="ExternalOutput")

    S_part = nc.dram_tensor("S_part", [B, KL], F32)
    S_full = nc.dram_tensor("S_full", [B, KL], F32, addr_space="Shared")
    Uh_part = nc.dram_tensor("Uh_part", [B, KL], F32)
    Uh_my = nc.dram_tensor("Uh_my", [B // NCORES, KL], F32)

    rg = [list(range(NCORES))]

    from contextlib import ExitStack

    with tile.TileContext(nc) as tc, ExitStack() as ctx:
        # ---- persistent pools ----
        persist = ctx.enter_context(tc.tile_pool(name="persist", bufs=1))
        uhat = persist.tile([128, NG, NCB, KL], F32)       # 128KB/partition
        srep = persist.tile([128, NCB, KL], F32)           # S replicated per (i_sub,b)
        asum = persist.tile([128, NCB, NG, 32], F32)       # A_sum: [p,(cb,g,k)]
        cc = persist.tile([128, NCB, NG, 32], F32)         # softmax C
        ones_sb = persist.tile([128, NCB, B], F32)
        small = ctx.enter_context(tc.tile_pool(name="small", bufs=1))
        ps_uh_pool = ctx.enter_context(
            tc.tile_pool(name="psuh", bufs=1, space="PSUM")
        )

        nc.sync.dma_start(
            out=ones_sb[:],
            in_=Ones[:, :, :].rearrange("c p b -> p c b"),
        )

        # warm the PE's view of ones_sb so later matmuls need no extra wait
        ps_uh = ps_uh_pool.tile([B, KL], F32)
        nc.tensor.matmul(
            ps_uh[0:1, 0:1],
            lhsT=ones_sb[:, 0, 0:1],
            rhs=ones_sb[:, 0, 0:1],
            start=True,
            stop=True,
        )

        # ================= phase A =================
        # Order: (1) S-partial matmuls over a first pass of WUL DMAs, kick off
        # the AllReduce; (2) all U_hat matmuls run UNDER the collective.
        with (
            tc.tile_pool(name="wul", bufs=2) as wul_pool,
            tc.tile_pool(name="wul2", bufs=3) as wul2_pool,
            tc.tile_pool(name="psu", bufs=2, space="PSUM") as psu_pool,
            tc.tile_pool(name="pss", bufs=1, space="PSUM") as pss_pool,
        ):
            ps_s = pss_pool.tile([B, KL], F32)
            for g in range(NG):
                wul_g = wul_pool.tile([128, KL + B], F32)
                nc.sync.dma_start(out=wul_g[:], in_=WUL[g, :, 0 : KL + B])
                wr_g = wul_g[:, 0:KL]
                ult_g = wul_g[:, KL : KL + B]
                for nch in range(2):
                    nc.tensor.matmul(
                        ps_s[:, nch * 512 : (nch + 1) * 512],
                        lhsT=ult_g,
                        rhs=wr_g[:, nch * 512 : (nch + 1) * 512],
                        start=(g == 0),
                        stop=(g == NG - 1),
                    )
            s_sb = small.tile([B, KL], F32, tag="stage4k")
            nc.scalar.copy(out=s_sb[:], in_=ps_s[:])
            nc.sync.dma_start(out=S_part[:, :], in_=s_sb[:])
            nc.gpsimd.collective_compute(
                "AllReduce",
                OP.add,
                replica_groups=rg,
                ins=[S_part[:, :]],
                outs=[S_full[:, :]],
            )

            # U_hat matmuls — second WUL pass, overlapped with the collective
            for g in range(NG):
                wul_g = wul2_pool.tile([128, 1600], F32)
                nc.sync.dma_start(out=wul_g[:], in_=WUL[g])
                wr_g = wul_g[:, 0:KL]
                for cb in range(NCB):
                    lb = wul_g[:, KL + B + 128 * cb : KL + B + 128 * (cb + 1)]
                    psu = psu_pool.tile([128, KL], F32)
                    for nch in range(2):
                        nc.tensor.matmul(
                            psu[:, nch * 512 : (nch + 1) * 512],
                            lhsT=lb,
                            rhs=wr_g[:, nch * 512 : (nch + 1) * 512],
                            start=True,
                            stop=True,
                        )
                    nc.scalar.copy(out=uhat[:, g, cb, :], in_=psu[:])

            # ---- replicate S across i_sub via PE broadcast matmuls ----
            bc_sb = small.tile([B, NCB, 128], F32)
            nc.sync.dma_start(
                out=bc_sb[:], in_=Bcast[:, :, :].rearrange("c b m -> b c m")
            )
            sf_sb = small.tile([B, KL], F32)
            nc.sync.dma_start(out=sf_sb[:], in_=S_full[:, :])
            for cb in range(NCB):
                ps_r = psu_pool.tile([128, KL], F32, tag="psu")
                for nch in range(2):
                    nc.tensor.matmul(
                        ps_r[:, nch * 512 : (nch + 1) * 512],
                        lhsT=bc_sb[:, cb, :],
                        rhs=sf_sb[:, nch * 512 : (nch + 1) * 512],
                        start=True,
                        stop=True,
                    )
                nc.scalar.copy(out=srep[:, cb, :], in_=ps_r[:])

        # ========= phases B+C interleaved per batch-chunk =========
        # B: A_sum = sum_l U_hat*S_rep, softmax over k (DVE owns reduces +
        # softmax, GPSIMD takes most B-multiplies).  C: tmp2 = U_hat*C and
        # PE block-ones partition-reduce into ps_uh.  Interleaving per cb
        # lets C(cb) overlap B(cb+1).  tmp pools open after phase A so they
        # reuse its freed SBUF.
        tmp_pool = ctx.enter_context(tc.tile_pool(name="tmp", bufs=4))
        for cb in range(NCB):
            for g in range(NG):
                tmp = tmp_pool.tile([128, 32, 32], F32, tag="tmpB")
                eng = nc.vector if ((g * NCB + cb) % 3 == 0) else nc.gpsimd
                eng.tensor_tensor(
                    tmp[:],
                    uhat[:, g, cb, :].rearrange("p (k l) -> p k l", l=32),
                    srep[:, cb, :].rearrange("p (k l) -> p k l", l=32),
                    OP.mult,
                )
                nc.vector.tensor_reduce(
                    asum[:, cb, g, :], tmp[:], axis=AX.X, op=OP.add
                )
            # softmax over k for this cb
            mx = small.tile([128, NG], F32)
            nc.vector.tensor_reduce(mx[:], asum[:, cb], axis=AX.X, op=OP.max)
            zs = small.tile([128, NG, 32], F32)
            nc.vector.tensor_tensor(
                zs[:], asum[:, cb], _bcast(mx[:], 32), OP.subtract
            )
            ex = small.tile([128, NG, 32], F32)
            nc.scalar.activation(ex[:], zs[:], AF.Exp, scale=1.0 / ATT)
            sm = small.tile([128, NG], F32)
            nc.vector.tensor_reduce(sm[:], ex[:], axis=AX.X, op=OP.add)
            rc = small.tile([128, NG], F32)
            nc.vector.reciprocal(rc[:], sm[:])
            nc.vector.tensor_tensor(cc[:, cb], ex[:], _bcast(rc[:], 32), OP.mult)

            # ---- C-work for this cb ----
            for g in range(NG):
                tmp2 = tmp_pool.tile([128, 32, 32], F32, tag="tmpC")
                eng = nc.gpsimd if ((g * NCB + cb) % 3 == 0) else nc.vector
                eng.tensor_tensor(
                    tmp2[:],
                    uhat[:, g, cb, :].rearrange("p (k l) -> p k l", l=32),
                    _bcast(cc[:, cb, g, :], 32),
                    OP.mult,
                )
                for nch in range(2):
                    nc.tensor.matmul(
                        ps_uh[:, nch * 512 : (nch + 1) * 512],
                        lhsT=ones_sb[:, cb, :],
                        rhs=tmp2[:].rearrange("p a b -> p (a b)")[
                            :, nch * 512 : (nch + 1) * 512
                        ],
                        start=(g == 0 and cb == 0),
                        stop=(g == NG - 1 and cb == NCB - 1),
                    )

        uh_sb = small.tile([B, KL], F32, tag="stage4k")
        nc.scalar.copy(out=uh_sb[:], in_=ps_uh[:])
        nc.sync.dma_start(out=Uh_part[:, :], in_=uh_sb[:])

        # ================= phase D: ReduceScatter + squash =================
        nc.gpsimd.collective_compute(
            "ReduceScatter",
            OP.add,
            replica_groups=rg,
            ins=[Uh_part[:, :]],
            outs=[Uh_my[:, :]],
        )
        nb = B // NCORES  # 8
        um = small.tile([nb, NH, DH], F32)
        nc.sync.dma_start(
            out=um[:], in_=Uh_my[:, :].rearrange("b (k l) -> b k l", l=DH)
        )
        sq = small.tile([nb, NH, DH], F32, tag="sq")
        nc.vector.tensor_tensor(sq[:], um[:], um[:], OP.mult)
        n2 = small.tile([nb, NH], F32)
        nc.vector.tensor_reduce(n2[:], sq[:], axis=AX.X, op=OP.add)
        nrm = small.tile([nb, NH], F32)
        nc.scalar.activation(nrm[:], n2[:], AF.Sqrt)
        ncl = small.tile([nb, NH], F32)
        nc.vector.tensor_scalar_min(ncl[:], nrm[:], 60.0)
        en = small.tile([nb, NH], F32)
        nc.scalar.activation(en[:], ncl[:], AF.Exp)
        re = small.tile([nb, NH], F32)
        nc.vector.reciprocal(re[:], en[:])
        one_t = small.tile([nb, NH], F32)
        nc.vector.memset(one_t[:], 1.0)
        f1 = small.tile([nb, NH], F32)
        nc.vector.tensor_tensor(f1[:], one_t[:], re[:], OP.subtract)
        nd = small.tile([nb, NH], F32)
        nc.vector.tensor_scalar_add(nd[:], nrm[:], EPS)
        rn = small.tile([nb, NH], F32)
        nc.vector.reciprocal(rn[:], nd[:])
        fac = small.tile([nb, NH], F32)
        nc.vector.tensor_tensor(fac[:], f1[:], rn[:], OP.mult)
        ov = small.tile([nb, NH, DH], F32, tag="sq")
        nc.vector.tensor_tensor(ov[:], um[:], _bcast(fac[:], DH), OP.mult)
        nc.sync.dma_start(
            out=out_sh[:, :], in_=ov[:].rearrange("b k l -> b (k l)")
        )

    nc.finalize()
    return nc


def host_prep(U_l, W):
    """Layout-only preprocessing of the full inputs into per-core in_maps."""
    U_l = np.asarray(U_l, dtype=np.float32)
    W = np.asarray(W, dtype=np.float32)
    # Ones[cb, 16*i_sub+br, b'] = 1 iff b' == 16*cb + br  (partition-sum matrix)
    ones = np.zeros((NCB, 128, B), dtype=np.float32)
    for cb in range(NCB):
        for i_sub in range(8):
            ones[cb, 16 * i_sub : 16 * (i_sub + 1), 16 * cb : 16 * (cb + 1)] = np.eye(
                16, dtype=np.float32
            )
    # Bcast[cb, b, 16*i_sub+br] = 1 iff b == 16*cb+br (partition replication)
    bcast = np.zeros((NCB, B, 128), dtype=np.float32)
    for cb in range(NCB):
        for i_sub in range(8):
            bcast[cb, 16 * cb : 16 * (cb + 1), 16 * i_sub : 16 * (i_sub + 1)] = np.eye(
                16, dtype=np.float32
            )
    in_maps = []
    for c in range(NCORES):
        i0 = c * ILOC
        Wsh = W[i0 : i0 + ILOC]                   # (64, 32, 16, 32)
        # Wr[g, 16*i_sub+j, 32*k+l] = W[i0+8g+i_sub, k, j, l]
        Wr = np.ascontiguousarray(
            Wsh.reshape(NG, 8, NH, DL, DH).transpose(0, 1, 3, 2, 4)
        ).reshape(NG, 128, KL)
        # UlT[g, 16*i_sub+j, b] = U_l[b, i0+8g+i_sub, j]
        Ush = U_l[:, i0 : i0 + ILOC, :]           # (64, 64, 16)
        UlT = np.ascontiguousarray(
            Ush.reshape(B, NG, 8, DL).transpose(1, 2, 3, 0)
        ).reshape(NG, 128, B)
        # Lb[g, cb, 16*i_sub+j, 16*i_sub+br] = U_l[16cb+br, i0+8g+i_sub, j]
        Lb = np.zeros((NG, NCB, 128, 128), dtype=np.float32)
        blocks = UlT.reshape(NG, 8, DL, NCB, 16)  # [g, i_sub, j, cb, br]
        for i_sub in range(8):
            Lb[:, :, 16 * i_sub : 16 * i_sub + DL, 16 * i_sub : 16 * (i_sub + 1)] = (
                blocks[:, i_sub].transpose(0, 2, 1, 3)
            )
        WUL = np.concatenate(
            [Wr, UlT, Lb.transpose(0, 2, 1, 3).reshape(NG, 128, NCB * 128)],
            axis=2,
        )
        in_maps.append({"WUL": WUL, "Ones": ones, "Bcast": bcast})
    return in_maps


def _build_executable(nc):
    """Build (once) a jitted shard_map'd callable around the compiled NEFF —
    mirrors concourse.bass2jax.run_bass_via_pjrt but reusable across calls
    without retracing."""
    import jax
    from jax.sharding import Mesh, PartitionSpec
    from jax.experimental.shard_map import shard_map
    from concourse import bass2jax

    bass2jax.install_neuronx_cc_hook()
    partition_name = nc.partition_id_tensor.name if nc.partition_id_tensor else None
    in_names, out_names, out_avals, zero_outs = [], [], [], []
    for alloc in nc.m.functions[0].allocations:
        if not isinstance(alloc, mybir.MemoryLocationSet):
            continue
        name = alloc.memorylocations[0].name
        if alloc.kind == "ExternalInput":
            if name != partition_name:
                in_names.append(name)
        elif alloc.kind == "ExternalOutput":
            shape = tuple(alloc.tensor_shape)
            dtype = mybir.dt.np(alloc.dtype)
            out_names.append(name)
            out_avals.append(jax.core.ShapedArray(shape, dtype))
            zero_outs.append(np.zeros(shape, dtype))
    n_params = len(in_names)
    n_outs = len(out_avals)
    all_names = list(in_names) + out_names
    if partition_name is not None:
        all_names.append(partition_name)

    def _body(*args):
        operands = list(args)
        if partition_name is not None:
            operands.append(bass2jax.partition_id_tensor())
        outs = bass2jax._bass_exec_p.bind(
            *operands,
            out_avals=tuple(out_avals),
            in_names=tuple(all_names),
            out_names=tuple(out_names),
            lowering_input_output_aliases=(),
            sim_require_finite=True,
            sim_require_nnan=True,
            nc=nc,
        )
        return tuple(outs)

    devices = jax.devices()[:NCORES]
    mesh = Mesh(np.asarray(devices), ("core",))
    fn = jax.jit(
        shard_map(
            _body,
            mesh=mesh,
            in_specs=(PartitionSpec("core"),) * (n_params + n_outs),
            out_specs=(PartitionSpec("core"),) * len(out_names),
            check_rep=False,
        ),
        donate_argnums=tuple(range(n_params, n_params + n_outs)),
        keep_unused=True,
    )

    def run(in_maps):
        import jax as _jax

        concat_in = [
            np.concatenate(
                [np.asarray(in_maps[c][nm]) for c in range(NCORES)], axis=0
            )
            for nm in in_names
        ]
        zeros = [
            np.zeros((NCORES * z.shape[0], *z.shape[1:]), z.dtype)
            for z in zero_outs
        ]
        out_arrs = fn(*concat_in, *zeros)
        out_arrs = [np.asarray(a) for a in _jax.block_until_ready(out_arrs)]
        return [
            {
                nm: out_arrs[i].reshape(NCORES, *out_avals[i].shape)[c]
                for i, nm in enumerate(out_names)
            }
            for c in range(NCORES)
        ]

    return run


def kernel(U_l, W):
    if "run" not in _CACHE:
        nc = build_program()
        _CACHE["nc"] = nc
        _CACHE["run"] = _build_executable(nc)
    in_maps = host_prep(U_l, W)
    results = _CACHE["run"](in_maps)
    out = np.concatenate(
        [results[c]["out_sh"].reshape(B // NCORES, NH, DH) for c in range(NCORES)],
        axis=0,
    )
    return out



# revision 17
# speedup vs baseline: 324.1788x; 324.1788x over previous
"""FCCaps (EfficientCapsNet fully-connected capsule routing) Trainium2 kernel.

Problem:  U_l (64, 512, 16) f32, W (512, 32, 16, 32) f32
    U_hat = einsum('bij,ikjl->bikl', U_l, W)          (B, n_l, n_h, d_h)
    A     = einsum('bikl,bhkl->bhik', U_hat, U_hat)
    C     = softmax(sum_i A / sqrt(d_h), axis=-1)      (B, n_l, n_h)
    U_h   = einsum('bikl,bik->bkl', U_hat, C)          (B, n_h, d_h)
    out   = squash(U_h)

Key algebraic identity used:
    A_sum[b,h,k] = sum_l U_hat[b,h,k,l] * S[b,k,l],  S = sum_i U_hat
so the (B, n_l, n_l, n_h) agreement tensor is never materialized.

Distribution: shard n_l (the i axis) 8 ways.  Each core computes U_hat for its
64 i's and all 64 batches, a partial S (AllReduce in fp16, 128KB), local
softmax (k is fully local), partial U_h (ReduceScatter over batch), then
squash of its own 8-batch output slice.  Host concatenates the 8 slices.

All PE matmuls run in float32r (1 cycle/row vs fp32's 4 for moving dims
>= 512; measured max rel err 1.6e-4 per 128-deep dot on HW, far inside the
2e-2 gate).  The S AllReduce runs in fp16 (halves payload; S only feeds the
softmax logits).

build_program(reps=K) emits the body K times back-to-back (per-iteration
DRAM scratch + SBUF pools) for differential wall-clock timing of the true
on-device execution time: hw_ns = (T(K2) - T(K1)) / (K2 - K1) cancels the
per-dispatch RPC/launch constant of the axon tunnel.

Host-side prep is layout-only (transpose/reshape/zero-fill of inputs).
"""

import sys

sys.path.insert(0, "/opt/trn_rl_repo")

import numpy as np

import concourse.bass as bass
import concourse.mybir as mybir
import concourse.tile as tile
from concourse import bacc
from concourse.bass_utils import run_bass_kernel_spmd

F32 = mybir.dt.float32
F32R = mybir.dt.float32r
F16 = mybir.dt.float16
BF16 = mybir.dt.bfloat16
AX = mybir.AxisListType
OP = mybir.AluOpType
AF = mybir.ActivationFunctionType

B, NL, NH, DL, DH = 64, 512, 32, 16, 32
NCORES = 8
ILOC = NL // NCORES          # 64 i's per core
NG = ILOC // 8               # 8 groups of 8 i_sub
NCB = B // 16                # 4 batch chunks of 16
KL = NH * DH                 # 1024
ATT = 5.656854249492381      # sqrt(d_h)
EPS = 1e-20

_CACHE = {}


def _bcast(ap, n):
    """Append a step-0 innermost dim (read-broadcast) to an AP."""
    return bass.AP(ap.tensor, ap.offset, list(ap.ap) + [[0, n]])


def _ins0(ap, n):
    """Insert a step-0 dim right after the partition dim (read-broadcast)."""
    a = list(ap.ap)
    return bass.AP(ap.tensor, ap.offset, a[:1] + [[0, n]] + a[1:])


def build_program(reps=1):
    nc = bacc.Bacc(
        "TRN2",
        target_bir_lowering=False,
        debug=False,
        enable_asserts=False,
        num_devices=NCORES,
    )

    # WUL[g] = bf16 concat(Wr_hi (1024), Wr_lo (1024), UlT_hi (64), UlT_lo
    # (64), Lb_hi (4x128), Lb_lo (4x128)) so one DMA covers all matmul
    # operands of a group.  hi/lo are a bf16 split (x = hi + lo to ~2^-18):
    # the S and U_hat matmuls run 3 bf16 passes (hi*hi + hi*lo + lo*hi) at
    # 1 cycle/row, giving near-fp32 precision at bf16 speed -- a single f32r
    # pass on the quadratically-amplified U_hat path measured 2.1e-2 rel err,
    # over the gate.
    WUL = nc.dram_tensor("WUL", [NG, 128, 3200], BF16, kind="ExternalInput")
    Ones = nc.dram_tensor("Ones", [NCB, 128, B], F32, kind="ExternalInput")
    # Bcast[cb][b, i_sub*16+br] = (b == 16*cb+br): PE-side partition broadcast
    # of S across the 8 i_sub replicas (fp16 to match the fp16 S path)
    Bcast = nc.dram_tensor("Bcast", [NCB, B, 128], F16, kind="ExternalInput")
    out_sh = nc.dram_tensor("out_sh", [B // NCORES, KL], F32, kind="ExternalOutput")

    rg = [list(range(NCORES))]

    from contextlib import ExitStack

    with tile.TileContext(nc) as tc, ExitStack() as ctx:
        hoist = ctx.enter_context(tc.tile_pool(name="hoist", bufs=1))
        ones_sb = hoist.tile([128, NCB, B], F32R)

        nc.sync.dma_start(
            out=ones_sb[:],
            in_=Ones[:, :, :].rearrange("c p b -> p c b").bitcast(F32R),
        )

        # warm the PE's view of ones_sb so later matmuls need no extra wait
        # (plain fp32: 1-wide f32r matmuls violate s3d3_mm_fp32r_restrictions)
        with tc.tile_pool(name="psw", bufs=1, space="PSUM") as ps_w_pool:
            ps_w = ps_w_pool.tile([1, 8], F32)
            nc.tensor.matmul(
                ps_w[0:1, 0:1],
                lhsT=ones_sb[:, 0, 0:1].bitcast(F32),
                rhs=ones_sb[:, 0, 0:1].bitcast(F32),
                start=True,
                stop=True,
            )

        # rotating engines for PSUM->SBUF evacuation copies.  Never gpsimd:
        # the collectives are issued from the Pool queue and block it for
        # their full duration in program order, so any Pool copy emitted
        # while the AllReduce is outstanding stalls the whole pipeline.
        def rot_copy(idx, out, in_):
            if idx % 2 == 0:
                nc.scalar.copy(out=out, in_=in_)
            else:
                nc.vector.tensor_copy(out=out, in_=in_)

        for it in range(reps):
            S_part = nc.dram_tensor(f"S_part{it}", [B, KL], F16)
            S_full = nc.dram_tensor(f"S_full{it}", [B, KL], F16, addr_space="Shared")
            Uh_part = nc.dram_tensor(f"Uh_part{it}", [B, KL], F32)
            Uh_my = nc.dram_tensor(f"Uh_my{it}", [B // NCORES, KL], F32)
            with ExitStack() as itx:
                persist = itx.enter_context(tc.tile_pool(name="persist", bufs=1))
                uhat = persist.tile([128, NG, NCB, KL], F32)   # 128KB/partition
                srep = persist.tile([128, NCB, KL], F32)       # S repl per (i_sub,b)
                asum = persist.tile([128, NCB, NG, 32], F32)   # A_sum: [p,(cb,g,k)]
                cc = persist.tile([128, NCB, NG, 32], F32)     # softmax C
                small = itx.enter_context(tc.tile_pool(name="small", bufs=1))
                ps_uh_pool = itx.enter_context(
                    tc.tile_pool(name="psuh", bufs=1, space="PSUM")
                )
                ps_uh = ps_uh_pool.tile([B, KL], F32)

                # ================= phase A =================
                # (1) S-partial matmuls over a first pass of WUL DMAs, kick off
                # the AllReduce; (2) all U_hat matmuls run UNDER the collective.
                with (
                    tc.tile_pool(name="wul", bufs=4) as wul_pool,
                    tc.tile_pool(name="wul2", bufs=2) as wul2_pool,
                    tc.tile_pool(name="psu", bufs=2, space="PSUM") as psu_pool,
                    tc.tile_pool(name="pss", bufs=1, space="PSUM") as pss_pool,
                ):
                    ps_s = pss_pool.tile([B, KL], F32)
                    for g in range(NG):
                        wul_g = wul_pool.tile([128, 2 * KL + 2 * B], BF16)
                        nc.sync.dma_start(
                            out=wul_g[:], in_=WUL[g, :, 0 : 2 * KL + 2 * B]
                        )
                        wrh = wul_g[:, 0:KL]
                        wrl = wul_g[:, KL : 2 * KL]
                        uth = wul_g[:, 2 * KL : 2 * KL + B]
                        utl = wul_g[:, 2 * KL + B : 2 * KL + 2 * B]
                        for p, (lt, wr) in enumerate(
                            ((uth, wrh), (uth, wrl), (utl, wrh))
                        ):
                            for nch in range(2):
                                nc.tensor.matmul(
                                    ps_s[:, nch * 512 : (nch + 1) * 512],
                                    lhsT=lt,
                                    rhs=wr[:, nch * 512 : (nch + 1) * 512],
                                    start=(g == 0 and p == 0),
                                    stop=(g == NG - 1 and p == 2),
                                )
                    s_sb = small.tile([B, KL], F16, tag="s16")
                    nc.scalar.copy(out=s_sb[:], in_=ps_s[:])
                    nc.sync.dma_start(out=S_part[:, :], in_=s_sb[:])
                    nc.gpsimd.collective_compute(
                        "AllReduce",
                        OP.add,
                        replica_groups=rg,
                        ins=[S_part[:, :]],
                        outs=[S_full[:, :]],
                    )

                    # U_hat matmuls — second WUL pass, under the collective
                    ci = 0
                    lb0 = 2 * KL + 2 * B
                    for g in range(NG):
                        wul_g = wul2_pool.tile([128, 3200], BF16)
                        nc.sync.dma_start(out=wul_g[:], in_=WUL[g])
                        wrh = wul_g[:, 0:KL]
                        wrl = wul_g[:, KL : 2 * KL]
                        for cb in range(NCB):
                            lbh = wul_g[:, lb0 + 128 * cb : lb0 + 128 * (cb + 1)]
                            lbl = wul_g[
                                :, lb0 + 512 + 128 * cb : lb0 + 512 + 128 * (cb + 1)
                            ]
                            psu = psu_pool.tile([128, KL], F32)
                            for p, (lb, wr) in enumerate(
                                ((lbh, wrh), (lbh, wrl), (lbl, wrh))
                            ):
                                for nch in range(2):
                                    nc.tensor.matmul(
                                        psu[:, nch * 512 : (nch + 1) * 512],
                                        lhsT=lb,
                                        rhs=wr[:, nch * 512 : (nch + 1) * 512],
                                        start=(p == 0),
                                        stop=(p == 2),
                                    )
                            rot_copy(ci, uhat[:, g, cb, :], psu[:])
                            ci += 1

                    # ---- replicate S across i_sub via PE broadcast matmuls ----
                    bc_sb = small.tile([B, NCB, 128], F16)
                    nc.sync.dma_start(
                        out=bc_sb[:], in_=Bcast[:, :, :].rearrange("c b m -> b c m")
                    )
                    sf_sb = small.tile([B, KL], F16, tag="s16b")
                    nc.sync.dma_start(out=sf_sb[:], in_=S_full[:, :])
                    for cb in range(NCB):
                        ps_r = psu_pool.tile([128, KL], F32, tag="psu")
                        for nch in range(2):
                            nc.tensor.matmul(
                                ps_r[:, nch * 512 : (nch + 1) * 512],
                                lhsT=bc_sb[:, cb, :],
                                rhs=sf_sb[:, nch * 512 : (nch + 1) * 512],
                                start=True,
                                stop=True,
                            )
                        rot_copy(cb, srep[:, cb, :], ps_r[:])

                # ========= phases B+C interleaved per batch-chunk =========
                # B: A_sum = sum_l U_hat*S_rep in g-pairs: Pool multiplies
                # [128,2,1024], DVE does the fused segmented reduce; softmax
                # over k on DVE+Act.  C: tmp2 = U_hat*C per g-pair (Pool takes
                # 3 of 4, DVE 1, emitted f32r) and PE block-ones
                # partition-reduce into ps_uh.  C(cb) overlaps B(cb+1).
                pb_pool = itx.enter_context(tc.tile_pool(name="pb", bufs=2))
                pc_pool = itx.enter_context(tc.tile_pool(name="pc", bufs=2))
                for cb in range(NCB):
                    for gp in range(NG // 2):
                        tb = pb_pool.tile([128, 2, KL], F32, tag="pb")
                        nc.gpsimd.tensor_tensor(
                            tb[:],
                            uhat[:, 2 * gp : 2 * gp + 2, cb, :],
                            _ins0(srep[:, cb, :], 2),
                            OP.mult,
                        )
                        nc.vector.tensor_reduce(
                            asum[:, cb, 2 * gp : 2 * gp + 2, :].rearrange(
                                "p g k -> p (g k)"
                            ),
                            tb[:].rearrange("p g (k l) -> p (g k) l", l=32),
                            axis=AX.X,
                            op=OP.add,
                        )
                    # softmax over k for this cb
                    mx = small.tile([128, NG], F32)
                    nc.vector.tensor_reduce(
                        mx[:], asum[:, cb], axis=AX.X, op=OP.max
                    )
                    zs = small.tile([128, NG, 32], F32)
                    nc.vector.tensor_tensor(
                        zs[:], asum[:, cb], _bcast(mx[:], 32), OP.subtract
                    )
                    ex = small.tile([128, NG, 32], F32)
                    nc.scalar.activation(ex[:], zs[:], AF.Exp, scale=1.0 / ATT)
                    sm = small.tile([128, NG], F32)
                    nc.vector.tensor_reduce(sm[:], ex[:], axis=AX.X, op=OP.add)
                    rc = small.tile([128, NG], F32)
                    nc.vector.reciprocal(rc[:], sm[:])
                    nc.vector.tensor_tensor(
                        cc[:, cb], ex[:], _bcast(rc[:], 32), OP.mult
                    )

                    # ---- C-work for this cb ----
                    for gp in range(NG // 2):
                        tmp2 = pc_pool.tile([128, 2, 32, 32], F32R, tag="pc")
                        eng = nc.gpsimd if gp < 3 else nc.vector
                        eng.tensor_tensor(
                            tmp2[:],
                            uhat[:, 2 * gp : 2 * gp + 2, cb, :].rearrange(
                                "p g (k l) -> p g k l", l=32
                            ),
                            _bcast(cc[:, cb, 2 * gp : 2 * gp + 2, :], 32),
                            OP.mult,
                        )
                        for nch in range(4):
                            nc.tensor.matmul(
                                ps_uh[:, (nch % 2) * 512 : (nch % 2 + 1) * 512],
                                lhsT=ones_sb[:, cb, :],
                                rhs=tmp2[:].rearrange("p a b c -> p (a b c)")[
                                    :, nch * 512 : (nch + 1) * 512
                                ],
                                start=(gp == 0 and cb == 0 and nch < 2),
                                stop=(
                                    gp == NG // 2 - 1
                                    and cb == NCB - 1
                                    and nch >= 2
                                ),
                            )

                uh_sb = small.tile([B, KL], F32, tag="stage4k")
                nc.scalar.copy(out=uh_sb[:], in_=ps_uh[:])
                nc.sync.dma_start(out=Uh_part[:, :], in_=uh_sb[:])

                # ============ phase D: ReduceScatter + squash ============
                nc.gpsimd.collective_compute(
                    "ReduceScatter",
                    OP.add,
                    replica_groups=rg,
                    ins=[Uh_part[:, :]],
                    outs=[Uh_my[:, :]],
                )
                # Squash on 128 partitions: Uh_my[b,(k,l)] as [(b,k/2),(k%2,l)]
                # so each DVE op touches 64 elems/partition instead of 1024.
                # factor = (1 - e^{-n})/(n+eps), n = ||x|| clamped at 60;
                # exp(-n) keeps the Act table on Exp (sqrt via DVE pow 0.5).
                um = small.tile([128, 2, DH], F32)
                nc.sync.dma_start(
                    out=um[:],
                    in_=Uh_my[:, :].rearrange(
                        "b (kh kl l) -> (b kh) kl l", kh=NH // 2, l=DH
                    ),
                )
                sq = small.tile([128, 2, DH], F32, tag="sq")
                nc.vector.tensor_tensor(sq[:], um[:], um[:], OP.mult)
                n2 = small.tile([128, 2], F32)
                nc.vector.tensor_reduce(n2[:], sq[:], axis=AX.X, op=OP.add)
                nrm = small.tile([128, 2], F32)
                nc.scalar.activation(nrm[:], n2[:], AF.Sqrt)
                ncl = small.tile([128, 2], F32)
                nc.vector.tensor_scalar_min(ncl[:], nrm[:], 60.0)
                en = small.tile([128, 2], F32)
                nc.scalar.activation(en[:], ncl[:], AF.Exp, scale=-1.0)
                one_t = small.tile([128, 2], F32)
                nc.vector.memset(one_t[:], 1.0)
                f1 = small.tile([128, 2], F32)
                nc.vector.tensor_tensor(f1[:], one_t[:], en[:], OP.subtract)
                nd = small.tile([128, 2], F32)
                nc.vector.tensor_scalar_add(nd[:], nrm[:], EPS)
                rn = small.tile([128, 2], F32)
                nc.vector.reciprocal(rn[:], nd[:])
                fac = small.tile([128, 2], F32)
                nc.vector.tensor_tensor(fac[:], f1[:], rn[:], OP.mult)
                ov = small.tile([128, 2, DH], F32, tag="sq")
                nc.vector.tensor_tensor(ov[:], um[:], _bcast(fac[:], DH), OP.mult)
                nc.sync.dma_start(
                    out=out_sh[:, :].rearrange(
                        "b (kh kl l) -> (b kh) kl l", kh=NH // 2, l=DH
                    ),
                    in_=ov[:],
                )

    nc.finalize()
    return nc


def host_prep(U_l, W):
    """Layout-only preprocessing of the full inputs into per-core in_maps."""
    U_l = np.asarray(U_l, dtype=np.float32)
    W = np.asarray(W, dtype=np.float32)
    # Ones[cb, 16*i_sub+br, b'] = 1 iff b' == 16*cb + br  (partition-sum matrix)
    ones = np.zeros((NCB, 128, B), dtype=np.float32)
    for cb in range(NCB):
        for i_sub in range(8):
            ones[cb, 16 * i_sub : 16 * (i_sub + 1), 16 * cb : 16 * (cb + 1)] = np.eye(
                16, dtype=np.float32
            )
    # Bcast[cb, b, 16*i_sub+br] = 1 iff b == 16*cb+br (partition replication)
    bcast = np.zeros((NCB, B, 128), dtype=np.float16)
    for cb in range(NCB):
        for i_sub in range(8):
            bcast[cb, 16 * cb : 16 * (cb + 1), 16 * i_sub : 16 * (i_sub + 1)] = np.eye(
                16, dtype=np.float16
            )
    import ml_dtypes

    bf16 = ml_dtypes.bfloat16

    def split(x):
        """bf16 hi/lo split: x ~= hi + lo with ~2^-18 relative error."""
        hi = x.astype(bf16)
        lo = (x - hi.astype(np.float32)).astype(bf16)
        return hi, lo

    in_maps = []
    for c in range(NCORES):
        i0 = c * ILOC
        Wsh = W[i0 : i0 + ILOC]                   # (64, 32, 16, 32)
        # Wr[g, 16*i_sub+j, 32*k+l] = W[i0+8g+i_sub, k, j, l]
        Wr = np.ascontiguousarray(
            Wsh.reshape(NG, 8, NH, DL, DH).transpose(0, 1, 3, 2, 4)
        ).reshape(NG, 128, KL)
        # UlT[g, 16*i_sub+j, b] = U_l[b, i0+8g+i_sub, j]
        Ush = U_l[:, i0 : i0 + ILOC, :]           # (64, 64, 16)
        UlT = np.ascontiguousarray(
            Ush.reshape(B, NG, 8, DL).transpose(1, 2, 3, 0)
        ).reshape(NG, 128, B)
        # Lb[g, cb, 16*i_sub+j, 16*i_sub+br] = U_l[16cb+br, i0+8g+i_sub, j]
        Lb = np.zeros((NG, NCB, 128, 128), dtype=np.float32)
        blocks = UlT.reshape(NG, 8, DL, NCB, 16)  # [g, i_sub, j, cb, br]
        for i_sub in range(8):
            Lb[:, :, 16 * i_sub : 16 * i_sub + DL, 16 * i_sub : 16 * (i_sub + 1)] = (
                blocks[:, i_sub].transpose(0, 2, 1, 3)
            )
        Lbf = Lb.transpose(0, 2, 1, 3).reshape(NG, 128, NCB * 128)
        Wrh, Wrl = split(Wr)
        Uth, Utl = split(UlT)
        Lbh, Lbl = split(Lbf)
        WUL = np.concatenate([Wrh, Wrl, Uth, Utl, Lbh, Lbl], axis=2)
        in_maps.append({"WUL": WUL, "Ones": ones, "Bcast": bcast})
    return in_maps


def _build_executable(nc):
    """Build (once) a jitted shard_map'd callable around the compiled NEFF —
    mirrors concourse.bass2jax.run_bass_via_pjrt but reusable across calls
    without retracing."""
    import jax
    from jax.sharding import Mesh, PartitionSpec
    from jax.experimental.shard_map import shard_map
    from concourse import bass2jax

    bass2jax.install_neuronx_cc_hook()
    partition_name = nc.partition_id_tensor.name if nc.partition_id_tensor else None
    in_names, out_names, out_avals, zero_outs = [], [], [], []
    for alloc in nc.m.functions[0].allocations:
        if not isinstance(alloc, mybir.MemoryLocationSet):
            continue
        name = alloc.memorylocations[0].name
        if alloc.kind == "ExternalInput":
            if name != partition_name:
                in_names.append(name)
        elif alloc.kind == "ExternalOutput":
            shape = tuple(alloc.tensor_shape)
            dtype = mybir.dt.np(alloc.dtype)
            out_names.append(name)
            out_avals.append(jax.core.ShapedArray(shape, dtype))
            zero_outs.append(np.zeros(shape, dtype))
    n_params = len(in_names)
    n_outs = len(out_avals)
    all_names = list(in_names) + out_names
    if partition_name is not None:
        all_names.append(partition_name)

    def _body(*args):
        operands = list(args)
        if partition_name is not None:
            operands.append(bass2jax.partition_id_tensor())
        outs = bass2jax._bass_exec_p.bind(
            *operands,
            out_avals=tuple(out_avals),
            in_names=tuple(all_names),
            out_names=tuple(out_names),
            lowering_input_output_aliases=(),
            sim_require_finite=True,
            sim_require_nnan=True,
            nc=nc,
        )
        return tuple(outs)

    devices = jax.devices()[:NCORES]
    mesh = Mesh(np.asarray(devices), ("core",))
    fn = jax.jit(
        shard_map(
            _body,
            mesh=mesh,
            in_specs=(PartitionSpec("core"),) * (n_params + n_outs),
            out_specs=(PartitionSpec("core"),) * len(out_names),
            check_rep=False,
        ),
        donate_argnums=tuple(range(n_params, n_params + n_outs)),
        keep_unused=True,
    )

    def run(in_maps):
        import jax as _jax

        concat_in = [
            np.concatenate(
                [np.asarray(in_maps[c][nm]) for c in range(NCORES)], axis=0
            )
            for nm in in_names
        ]
        zeros = [
            np.zeros((NCORES * z.shape[0], *z.shape[1:]), z.dtype)
            for z in zero_outs
        ]
        out_arrs = fn(*concat_in, *zeros)
        out_arrs = [np.asarray(a) for a in _jax.block_until_ready(out_arrs)]
        return [
            {
                nm: out_arrs[i].reshape(NCORES, *out_avals[i].shape)[c]
                for i, nm in enumerate(out_names)
            }
            for c in range(NCORES)
        ]

    return run


def kernel(U_l, W):
    if "run" not in _CACHE:
        nc = build_program()
        _CACHE["nc"] = nc
        _CACHE["run"] = _build_executable(nc)
    in_maps = host_prep(U_l, W)
    results = _CACHE["run"](in_maps)
    out = np.concatenate(
        [results[c]["out_sh"].reshape(B // NCORES, NH, DH) for c in range(NCORES)],
        axis=0,
    )
    return out
